# revision 1
# baseline (speedup 1.0000x reference)
"""DiffHead (differential attention) Trainium2 Bass kernel — V1 optimized.

Sharding: 8 cores = 4 batches x 2 heads. Each core computes, for its
(batch, head): projections QT/KT/V from x^T, causal-masked exp-scores in
"keys-on-partitions" orientation, and the unnormalized attention output
OT[e, q] = sum_k V[k,e] * exp(S[q,k]) along with row sums l[q] (softmax
denominators). Host normalizes, transposes, and combines the two heads:
out_b = softmax1 @ v - lam * softmax2 @ v.

V1 changes vs baseline:
- fp16 everywhere off the PE: e tiles, acc, masks are fp16 in SBUF so DVE
  tensor ops run in 4x mode (0.25 cycles/elem vs 1.0 for fp32).
- V computed directly in [keys, e] layout by swapping matmul operands
  (lhsT = x^T chunk, rhs = Wv chunk) — kills 16 PE transposes and 20 DVE
  copies per core.
- One x DMA per 512-token block (3D AP) instead of 8 chunk DMAs — cuts
  HWDGE/SEQ setup from 46 to 22 DMAs per rep.
- OT output stored as fp16 (halves output DMA bytes); host upconverts.

V2 changes vs V1:
- The softmax denominator is NOT reduced on device: the per-partition
  partial sums `acc` [128, T] (fp16) are DMA'd out directly and the host
  does the final 128-way sum. Kills the ones-matmul (PE), the [1,512]
  l copies (DVE), and their DMAs.
- qk PSUM->SBUF copy moved from ACT to DVE to balance engines.

V3 changes vs V2:
- Full (non-diagonal) score tiles are computed in PAIRS into one 2-bank
  PSUM tile [128, 2, 512] so exp runs once per pair on 1024 columns —
  28 ACT instructions instead of 40 (each ACT op pays ~220 cycles of
  SBUF-access init on top of the per-column work).
"""

import sys

sys.path.insert(0, "/opt/trn_rl_repo")

import numpy as np  # noqa: E402

import concourse.bass as bass  # noqa: E402,F401
import concourse.tile as tile  # noqa: E402
from concourse import bacc, mybir  # noqa: E402
from concourse.bass_utils import run_bass_kernel_spmd  # noqa: E402

T = 2048
C = 1024
D = 64  # head dim
E = 128  # v dim (2 * HEAD)
P = 128
NC = C // P  # 8 contraction chunks
QB = 512  # query block (matmul free dim)
NQB = T // QB  # 4
KTILES = T // P  # 16 key tiles
SCALE = 0.125  # 1/sqrt(64)
LOOKAHEAD = 3

F32 = mybir.dt.float32
F16 = mybir.dt.float16

_CACHE = {}


def _build_nc(loop_n=0):
    """Build the per-core program. loop_n > 0 wraps the body in an on-device
    loop (benchmarking only)."""
    nc = bacc.Bacc("TRN2", target_bir_lowering=False, debug=False)

    xt_d = nc.dram_tensor("xt", [C, T], F16, kind="ExternalInput")
    wqk_d = nc.dram_tensor("wqk", [C, 2 * D], F16, kind="ExternalInput")
    wv_d = nc.dram_tensor("wv", [C, E], F16, kind="ExternalInput")
    ot_d = nc.dram_tensor("ot", [E, T], F16, kind="ExternalOutput")
    ls_d = nc.dram_tensor("ls", [P, T], F16, kind="ExternalOutput")

    with tile.TileContext(nc) as tc:
        from contextlib import ExitStack

        with ExitStack() as ctx:
            cpool = ctx.enter_context(tc.tile_pool(name="const", bufs=1))
            pps = ctx.enter_context(tc.tile_pool(name="pps", bufs=2, space="PSUM"))
            stp = ctx.enter_context(tc.tile_pool(name="stp", bufs=2, space="PSUM"))
            otp = ctx.enter_context(tc.tile_pool(name="otp", bufs=2, space="PSUM"))
            wpool = ctx.enter_context(tc.tile_pool(name="work", bufs=6))
            opool = ctx.enter_context(tc.tile_pool(name="outs", bufs=3))

            xt_sb = cpool.tile([P, NC, T], F16)
            wqk_sb = cpool.tile([P, NC, 2 * D], F16)
            wv_sb = cpool.tile([P, NC, E], F16)
            qk_sb = cpool.tile([P, T], F16)  # rows 0:64 = QT, 64:128 = KT
            kt_sb = cpool.tile([D, T], F16)  # KT repositioned to partitions 0:64
            v_sb = cpool.tile([P, KTILES, E], F16)
            mask_f32 = cpool.tile([P, QB], F32, tag="maskf", name="maskf")
            masks = [
                cpool.tile([P, QB], F16, tag=f"mask{j}", name=f"mask{j}")
                for j in range(4)
            ]

            # one-time constants (outside the bench loop)
            # mask j: keep (1.0) iff key_local + 128*j <= query_local
            for j in range(4):
                nc.gpsimd.memset(mask_f32[:], 1.0)
                nc.gpsimd.affine_select(
                    out=mask_f32[:],
                    in_=mask_f32[:],
                    compare_op=mybir.AluOpType.is_ge,
                    fill=0.0,
                    base=-128 * j,
                    pattern=[[1, QB]],
                    channel_multiplier=-1,
                )
                nc.vector.tensor_copy(masks[j][:], mask_f32[:])

            xt_r = xt_d.rearrange("(n p) t -> p n t", p=P)

            def body():
                nc.sync.dma_start(
                    wqk_sb[:], wqk_d.rearrange("(n p) d -> p n d", p=P)
                )
                nc.sync.dma_start(wv_sb[:], wv_d.rearrange("(n p) d -> p n d", p=P))

                for tb in range(NQB):
                    ts_ = slice(tb * QB, (tb + 1) * QB)
                    # --- load this token block (single 3D DMA) ---
                    nc.sync.dma_start(xt_sb[:, :, ts_], xt_r[:, :, ts_])
                    # --- QK projection for this block ---
                    qkp = pps.tile([P, QB], F32, tag="proj", name="qkp")
                    for c in range(NC):
                        nc.tensor.matmul(
                            qkp[:], wqk_sb[:, c, :], xt_sb[:, c, ts_],
                            start=(c == 0), stop=(c == NC - 1),
                        )
                    nc.vector.tensor_copy(qk_sb[:, ts_], qkp[:])
                    # reposition KT (rows 64:128) to partitions 0:64
                    nc.sync.dma_start(kt_sb[:, ts_], qk_sb[D : 2 * D, ts_])
                    # --- V for the 4 key tiles of this block, direct layout.
                    # All 4 [128,128] tiles share one PSUM bank. ---
                    vp = pps.tile([P, 4, E], F32, tag="proj", name="vp")
                    for i in range(4):
                        k = 4 * tb + i
                        ks_ = slice(k * P, (k + 1) * P)
                        for c in range(NC):
                            nc.tensor.matmul(
                                vp[:, i, :], xt_sb[:, c, ks_], wv_sb[:, c, :],
                                start=(c == 0), stop=(c == NC - 1),
                            )
                    nc.vector.tensor_copy(
                        v_sb[:, 4 * tb : 4 * tb + 4, :], vp[:]
                    )

                    # --- attention for query block qb == tb ---
                    qb = tb
                    qs = ts_
                    nkt = 4 * (qb + 1)
                    ot_ps = otp.tile([P, QB], F32, tag="ot", name="ot_ps")
                    acc = wpool.tile([P, QB], F16, tag="acc", name="acc")
                    etiles = [None] * nkt
                    # valid query range start for each kt (diagonal trim):
                    # tile kt only has unmasked entries for q_local >= 128j.
                    qlo = [max(0, 128 * (kt - 4 * qb)) for kt in range(nkt)]

                    def emit_pv(kt, ot_ps=ot_ps, etiles=etiles, nkt=nkt, qlo=qlo):
                        lo = qlo[kt]
                        nc.tensor.matmul(
                            ot_ps[:, lo:], v_sb[:, kt, :], etiles[kt][:, lo:],
                            start=(kt == 0), stop=(kt == nkt - 1),
                            skip_group_check=True,
                        )

                    def post_exp(kt, acc=acc, etiles=etiles, qb=qb):
                        """mask (diagonal), accumulate into acc, emit pv."""
                        e = etiles[kt]
                        lo = max(0, 128 * (kt - 4 * qb))
                        j = kt - 4 * qb
                        if j >= 0:
                            # triangular boundary lives in a 128-col window
                            w = slice(lo, lo + P)
                            nc.vector.tensor_tensor(
                                e[:, w], e[:, w], masks[j][:, w],
                                mybir.AluOpType.mult,
                            )
                        if kt == 0:
                            nc.vector.tensor_copy(acc[:], e[:])
                        else:
                            nc.vector.tensor_add(
                                acc[:, lo:], acc[:, lo:], e[:, lo:]
                            )
                        if kt >= LOOKAHEAD:
                            emit_pv(kt - LOOKAHEAD)

                    # full tiles, two at a time: one exp per 1024 columns
                    for pi in range(2 * qb):
                        stp2 = stp.tile([P, 2, QB], F32, tag="st", name="st")
                        ep = wpool.tile([P, 2, QB], F16, tag="e", name="ep")
                        for h in range(2):
                            kt = 2 * pi + h
                            nc.tensor.matmul(
                                stp2[:, h, :],
                                kt_sb[:, kt * P : (kt + 1) * P],
                                qk_sb[:D, qs],
                                start=True, stop=True,
                            )
                            etiles[kt] = ep[:, h, :]
                        nc.scalar.activation(
                            ep[:], stp2[:],
                            mybir.ActivationFunctionType.Exp, scale=SCALE,
                        )
                        post_exp(2 * pi)
                        post_exp(2 * pi + 1)
                    # diagonal tiles, trimmed to the valid query range
                    for kt in range(4 * qb, nkt):
                        lo = qlo[kt]
                        st = stp.tile([P, 2, QB], F32, tag="st", name="st")
                        nc.tensor.matmul(
                            st[:, 0, lo:],
                            kt_sb[:, kt * P : (kt + 1) * P],
                            qk_sb[:D, tb * QB + lo : (tb + 1) * QB],
                            start=True, stop=True,
                        )
                        e = wpool.tile([P, QB], F16, tag="ed", name="ed")
                        etiles[kt] = e
                        nc.scalar.activation(
                            e[:, lo:], st[:, 0, lo:],
                            mybir.ActivationFunctionType.Exp, scale=SCALE,
                        )
                        post_exp(kt)
                    for kt in range(max(0, nkt - LOOKAHEAD), nkt):
                        emit_pv(kt)

                    oc = opool.tile([P, QB], F16, tag="oc", name="oc")
                    nc.vector.tensor_copy(oc[:], ot_ps[:])
                    nc.sync.dma_start(ot_d[:, qs], oc[:])
                    nc.sync.dma_start(ls_d[:, qs], acc[:])

            for _rep in range(max(1, loop_n)):
                body()

    nc.finalize()
    return nc


def _get_nc(loop_n=0):
    key = ("nc", loop_n)
    if key not in _CACHE:
        _CACHE[key] = _build_nc(loop_n)
    return _CACHE[key]


def _make_in_maps(inputs):
    x = np.asarray(inputs["x"], dtype=np.float32)
    Wq1 = np.asarray(inputs["Wq1"], dtype=np.float32)
    Wk1 = np.asarray(inputs["Wk1"], dtype=np.float32)
    Wq2 = np.asarray(inputs["Wq2"], dtype=np.float32)
    Wk2 = np.asarray(inputs["Wk2"], dtype=np.float32)
    Wv = np.asarray(inputs["Wv"], dtype=np.float32)
    B = x.shape[0]

    def _cvt(a):
        return np.ascontiguousarray(a).astype(np.float16)

    wqk1 = _cvt(np.concatenate([Wq1, Wk1], axis=1))
    wqk2 = _cvt(np.concatenate([Wq2, Wk2], axis=1))
    wv = _cvt(Wv)
    in_maps = []
    for core in range(8):
        b, h = core // 2, core % 2
        in_maps.append(
            {
                "xt": _cvt(x[b].T),
                "wqk": wqk1 if h == 0 else wqk2,
                "wv": wv,
            }
        )
    return in_maps, B


def _lam(inputs):
    lq1 = np.asarray(inputs["lambda_q1"], dtype=np.float32)
    lk1 = np.asarray(inputs["lambda_k1"], dtype=np.float32)
    lq2 = np.asarray(inputs["lambda_q2"], dtype=np.float32)
    lk2 = np.asarray(inputs["lambda_k2"], dtype=np.float32)
    layer_idx = np.float32(np.asarray(inputs["layer_idx"]))
    dyn_init = np.float32(0.8) - np.float32(0.6) * np.exp(
        np.float32(-0.3) * (layer_idx - np.float32(1.0))
    )
    return np.float32(np.mean(np.exp(lq1 * lk1) - np.exp(lq2 * lk2) + dyn_init))


def _combine(results, lam, B):
    out = np.empty((B, T, E), dtype=np.float32)
    for b in range(B):
        r1, r2 = results[2 * b], results[2 * b + 1]
        l1 = r1["ls"].astype(np.float32).sum(axis=0)  # [T]
        l2 = r2["ls"].astype(np.float32).sum(axis=0)
        o1 = r1["ot"].astype(np.float32) / l1  # [E, T]
        o2 = r2["ot"].astype(np.float32) / l2
        out[b] = (o1 - lam * o2).T
    return out


def run_cores(inputs, loop_n=0, **kwargs):
    """Run the SPMD kernel; returns (BassKernelResults, lam, B)."""
    in_maps, B = _make_in_maps(inputs)
    res = run_bass_kernel_spmd(
        _get_nc(loop_n), in_maps, core_ids=list(range(8)), **kwargs
    )
    return res, _lam(inputs), B


def kernel(**inputs) -> np.ndarray:
    res, lam, B = run_cores(inputs)
    return _combine(res.results, lam, B)



# revision 19
# speedup vs baseline: 1.2261x; 1.2261x over previous
"""DiffHead (differential attention) Trainium2 Bass kernel — V1 optimized.

Sharding: 8 cores = 4 batches x 2 heads. Each core computes, for its
(batch, head): projections QT/KT/V from x^T, causal-masked exp-scores in
"keys-on-partitions" orientation, and the unnormalized attention output
OT[e, q] = sum_k V[k,e] * exp(S[q,k]) along with row sums l[q] (softmax
denominators). Host normalizes, transposes, and combines the two heads:
out_b = softmax1 @ v - lam * softmax2 @ v.

V1 changes vs baseline:
- fp16 everywhere off the PE: e tiles, acc, masks are fp16 in SBUF so DVE
  tensor ops run in 4x mode (0.25 cycles/elem vs 1.0 for fp32).
- V computed directly in [keys, e] layout by swapping matmul operands
  (lhsT = x^T chunk, rhs = Wv chunk) — kills 16 PE transposes and 20 DVE
  copies per core.
- One x DMA per 512-token block (3D AP) instead of 8 chunk DMAs — cuts
  HWDGE/SEQ setup from 46 to 22 DMAs per rep.
- OT output stored as fp16 (halves output DMA bytes); host upconverts.

V2 changes vs V1:
- The softmax denominator is NOT reduced on device: the per-partition
  partial sums `acc` [128, T] (fp16) are DMA'd out directly and the host
  does the final 128-way sum. Kills the ones-matmul (PE), the [1,512]
  l copies (DVE), and their DMAs.
- qk PSUM->SBUF copy moved from ACT to DVE to balance engines.

V3 changes vs V2:
- Full (non-diagonal) score tiles are computed in PAIRS into one 2-bank
  PSUM tile [128, 2, 512] so exp runs once per pair on 1024 columns —
  28 ACT instructions instead of 40 (each ACT op pays ~220 cycles of
  SBUF-access init on top of the per-column work).

V5 changes vs V3 (score matmul row-group packing):
- Score matmuls have contraction D=64 — only half the PE array rows.
  Each pair of score tiles now runs CONCURRENTLY in the array via
  tile_position row tiling: the even tile uses PE rows 0:63 (K copy at
  partitions 0:63, Q at partitions 0:63) and the odd tile uses rows
  64:127 (K original at partitions 64:127 in qk_sb, plus a new Q copy
  at partitions 64:127 in qt2_sb). Concurrent row-group matmuls stream
  simultaneously (measured ~2x for 2-way on TRN2), halving score wall
  time from ~17408 to ~8704 PE cycles/rep. LDWEIGHTS of one group also
  overlaps the other group's matmul (per-subarray concurrency).
- Diagonal tiles are paired the same way (one 2-bank PSUM tile per
  diagonal pair, two trimmed exps).
- vp moved into the score PSUM pool (its slot's first bank), freeing a
  bank so the pool budget stays exactly 8 banks.

V9 changes vs V5 (DMA queue restructuring, ~4.5 us/rep measured):
- All four x-block transfers are prefetched up front on the SP hwdge
  queue. Previously block b+1's 3.2 us x transfer sat in the FIFO
  behind block b's end-of-block output stores (which wait on the last
  DVE ops of the block), stalling every block's QK projection.
- The latency-critical kt/qt2 SBUF-to-SBUF repositioning copies moved
  to the ACT hwdge queue so they are not queued behind the x
  prefetches. Output stores stay on SP (queued after all x transfers,
  where they block nothing).
"""

import sys

sys.path.insert(0, "/opt/trn_rl_repo")

import numpy as np  # noqa: E402

import concourse.bass as bass  # noqa: E402,F401
import concourse.tile as tile  # noqa: E402
from concourse import bacc, mybir  # noqa: E402
from concourse.bass_utils import run_bass_kernel_spmd  # noqa: E402

T = 2048
C = 1024
D = 64  # head dim
E = 128  # v dim (2 * HEAD)
P = 128
NC = C // P  # 8 contraction chunks
QB = 512  # query block (matmul free dim)
NQB = T // QB  # 4
KTILES = T // P  # 16 key tiles
SCALE = 0.125  # 1/sqrt(64)
LOOKAHEAD = 3

F32 = mybir.dt.float32
F16 = mybir.dt.float16

_CACHE = {}


def _build_nc(loop_n=0):
    """Build the per-core program. loop_n > 0 wraps the body in an on-device
    loop (benchmarking only)."""
    nc = bacc.Bacc("TRN2", target_bir_lowering=False, debug=False)

    xt_d = nc.dram_tensor("xt", [C, T], F16, kind="ExternalInput")
    wqk_d = nc.dram_tensor("wqk", [C, 2 * D], F16, kind="ExternalInput")
    wv_d = nc.dram_tensor("wv", [C, E], F16, kind="ExternalInput")
    ot_d = nc.dram_tensor("ot", [E, T], F16, kind="ExternalOutput")
    ls_d = nc.dram_tensor("ls", [P, T], F16, kind="ExternalOutput")

    with tile.TileContext(nc) as tc:
        from contextlib import ExitStack

        with ExitStack() as ctx:
            cpool = ctx.enter_context(tc.tile_pool(name="const", bufs=1))
            pps = ctx.enter_context(tc.tile_pool(name="pps", bufs=2, space="PSUM"))
            stp = ctx.enter_context(tc.tile_pool(name="stp", bufs=2, space="PSUM"))
            otp = ctx.enter_context(tc.tile_pool(name="otp", bufs=2, space="PSUM"))
            wpool = ctx.enter_context(tc.tile_pool(name="work", bufs=6))
            opool = ctx.enter_context(tc.tile_pool(name="outs", bufs=3))

            xt_sb = cpool.tile([P, NC, T], F16)
            wqk_sb = cpool.tile([P, NC, 2 * D], F16)
            wv_sb = cpool.tile([P, NC, E], F16)
            qk_sb = cpool.tile([P, T], F16)  # rows 0:64 = QT, 64:128 = KT
            kt_sb = cpool.tile([D, T], F16)  # KT repositioned to partitions 0:64
            qt2_sb = cpool.tile([P, T], F16)  # rows 64:128 = QT copy
            v_sb = cpool.tile([P, KTILES, E], F16)
            mask_f32 = cpool.tile([P, QB], F32, tag="maskf", name="maskf")
            masks = [
                cpool.tile([P, QB], F16, tag=f"mask{j}", name=f"mask{j}")
                for j in range(4)
            ]

            # one-time constants (outside the bench loop)
            # mask j: keep (1.0) iff key_local + 128*j <= query_local
            for j in range(4):
                nc.gpsimd.memset(mask_f32[:], 1.0)
                nc.gpsimd.affine_select(
                    out=mask_f32[:],
                    in_=mask_f32[:],
                    compare_op=mybir.AluOpType.is_ge,
                    fill=0.0,
                    base=-128 * j,
                    pattern=[[1, QB]],
                    channel_multiplier=-1,
                )
                nc.vector.tensor_copy(masks[j][:], mask_f32[:])

            xt_r = xt_d.rearrange("(n p) t -> p n t", p=P)

            def body():
                nc.sync.dma_start(
                    wqk_sb[:], wqk_d.rearrange("(n p) d -> p n d", p=P)
                )
                nc.sync.dma_start(wv_sb[:], wv_d.rearrange("(n p) d -> p n d", p=P))
                # Prefetch all four x blocks up front on the SP queue. The
                # kt/qt2 SBUF-SBUF copies ride the ACT hwdge queue instead,
                # so they are not stuck behind the x prefetches, and block
                # b+1's x transfer is no longer stuck behind block b's
                # end-of-block output stores (which stay on SP, queued after
                # all x transfers).
                for tb in range(NQB):
                    ts_ = slice(tb * QB, (tb + 1) * QB)
                    nc.sync.dma_start(xt_sb[:, :, ts_], xt_r[:, :, ts_])

                for tb in range(NQB):
                    ts_ = slice(tb * QB, (tb + 1) * QB)
                    # --- QK projection for this block ---
                    qkp = pps.tile([P, QB], F32, tag="proj", name="qkp")
                    for c in range(NC):
                        nc.tensor.matmul(
                            qkp[:], wqk_sb[:, c, :], xt_sb[:, c, ts_],
                            start=(c == 0), stop=(c == NC - 1),
                        )
                    nc.vector.tensor_copy(qk_sb[:, ts_], qkp[:])
                    # reposition KT (rows 64:128) to partitions 0:64 and QT
                    # (rows 0:64) to partitions 64:128 for row-group packing
                    nc.scalar.dma_start(kt_sb[:, ts_], qk_sb[D : 2 * D, ts_])
                    nc.scalar.dma_start(qt2_sb[D : 2 * D, ts_], qk_sb[:D, ts_])
                    # --- V for the 4 key tiles of this block, direct layout.
                    # All 4 [128,128] tiles share one PSUM bank (slot shared
                    # with the score pool to stay within 8 banks). ---
                    vp = stp.tile([P, 4, E], F32, tag="st", name="vp")
                    for i in range(4):
                        k = 4 * tb + i
                        ks_ = slice(k * P, (k + 1) * P)
                        for c in range(NC):
                            nc.tensor.matmul(
                                vp[:, i, :], xt_sb[:, c, ks_], wv_sb[:, c, :],
                                start=(c == 0), stop=(c == NC - 1),
                            )
                    nc.vector.tensor_copy(
                        v_sb[:, 4 * tb : 4 * tb + 4, :], vp[:]
                    )

                    # --- attention for query block qb == tb ---
                    qb = tb
                    qs = ts_
                    nkt = 4 * (qb + 1)
                    ot_ps = otp.tile([P, QB], F32, tag="ot", name="ot_ps")
                    acc = wpool.tile([P, QB], F16, tag="acc", name="acc")
                    etiles = [None] * nkt
                    # valid query range start for each kt (diagonal trim):
                    # tile kt only has unmasked entries for q_local >= 128j.
                    qlo = [max(0, 128 * (kt - 4 * qb)) for kt in range(nkt)]

                    def emit_pv(kt, ot_ps=ot_ps, etiles=etiles, nkt=nkt, qlo=qlo):
                        lo = qlo[kt]
                        nc.tensor.matmul(
                            ot_ps[:, lo:], v_sb[:, kt, :], etiles[kt][:, lo:],
                            start=(kt == 0), stop=(kt == nkt - 1),
                            skip_group_check=True,
                        )

                    def post_exp(kt, acc=acc, etiles=etiles, qb=qb):
                        """mask (diagonal), accumulate into acc, emit pv."""
                        e = etiles[kt]
                        lo = max(0, 128 * (kt - 4 * qb))
                        j = kt - 4 * qb
                        if j >= 0:
                            # triangular boundary lives in a 128-col window
                            w = slice(lo, lo + P)
                            nc.vector.tensor_tensor(
                                e[:, w], e[:, w], masks[j][:, w],
                                mybir.AluOpType.mult,
                            )
                        if kt == 0:
                            nc.vector.tensor_copy(acc[:], e[:])
                        else:
                            nc.vector.tensor_add(
                                acc[:, lo:], acc[:, lo:], e[:, lo:]
                            )
                        if kt >= LOOKAHEAD:
                            emit_pv(kt - LOOKAHEAD)

                    # Score tiles two at a time: even tile on PE rows 0:63
                    # (kt_sb K copy + qk_sb Q, both at partitions 0:63), odd
                    # tile on PE rows 64:127 (qk_sb K original + qt2_sb Q
                    # copy, both at partitions 64:127). tile_position is
                    # auto-derived from the operands' base partition, and the
                    # two matmuls run concurrently in the array.
                    # full (non-diagonal) pairs: one exp per 1024 columns
                    for pi in range(2 * qb):
                        stp2 = stp.tile([P, 2, QB], F32, tag="st", name="st")
                        ep = wpool.tile([P, 2, QB], F16, tag="e", name="ep")
                        kt0, kt1 = 2 * pi, 2 * pi + 1
                        nc.tensor.matmul(
                            stp2[:, 0, :],
                            kt_sb[:, kt0 * P : (kt0 + 1) * P],
                            qk_sb[:D, qs],
                            start=True, stop=True,
                        )
                        nc.tensor.matmul(
                            stp2[:, 1, :],
                            qk_sb[D : 2 * D, kt1 * P : (kt1 + 1) * P],
                            qt2_sb[D : 2 * D, qs],
                            start=True, stop=True,
                        )
                        etiles[kt0] = ep[:, 0, :]
                        etiles[kt1] = ep[:, 1, :]
                        nc.scalar.activation(
                            ep[:], stp2[:],
                            mybir.ActivationFunctionType.Exp, scale=SCALE,
                        )
                        post_exp(kt0)
                        post_exp(kt1)
                    # diagonal tiles, trimmed to the valid query range,
                    # packed pairwise the same way (two trimmed exps)
                    for dj in range(2):
                        ktA = 4 * qb + 2 * dj
                        ktB = ktA + 1
                        loA, loB = qlo[ktA], qlo[ktB]
                        st = stp.tile([P, 2, QB], F32, tag="st", name="st")
                        ed = wpool.tile([P, 2, QB], F16, tag="e", name="ed")
                        nc.tensor.matmul(
                            st[:, 0, loA:],
                            kt_sb[:, ktA * P : (ktA + 1) * P],
                            qk_sb[:D, tb * QB + loA : (tb + 1) * QB],
                            start=True, stop=True,
                        )
                        nc.tensor.matmul(
                            st[:, 1, loB:],
                            qk_sb[D : 2 * D, ktB * P : (ktB + 1) * P],
                            qt2_sb[D : 2 * D, tb * QB + loB : (tb + 1) * QB],
                            start=True, stop=True,
                        )
                        etiles[ktA] = ed[:, 0, :]
                        etiles[ktB] = ed[:, 1, :]
                        nc.scalar.activation(
                            ed[:, 0, loA:], st[:, 0, loA:],
                            mybir.ActivationFunctionType.Exp, scale=SCALE,
                        )
                        nc.scalar.activation(
                            ed[:, 1, loB:], st[:, 1, loB:],
                            mybir.ActivationFunctionType.Exp, scale=SCALE,
                        )
                        post_exp(ktA)
                        post_exp(ktB)
                    for kt in range(max(0, nkt - LOOKAHEAD), nkt):
                        emit_pv(kt)

                    oc = opool.tile([P, QB], F16, tag="oc", name="oc")
                    nc.vector.tensor_copy(oc[:], ot_ps[:])
                    nc.sync.dma_start(ot_d[:, qs], oc[:])
                    nc.sync.dma_start(ls_d[:, qs], acc[:])

            for _rep in range(max(1, loop_n)):
                body()

    nc.finalize()
    return nc


def _get_nc(loop_n=0):
    key = ("nc", loop_n)
    if key not in _CACHE:
        _CACHE[key] = _build_nc(loop_n)
    return _CACHE[key]


def _make_in_maps(inputs):
    x = np.asarray(inputs["x"], dtype=np.float32)
    Wq1 = np.asarray(inputs["Wq1"], dtype=np.float32)
    Wk1 = np.asarray(inputs["Wk1"], dtype=np.float32)
    Wq2 = np.asarray(inputs["Wq2"], dtype=np.float32)
    Wk2 = np.asarray(inputs["Wk2"], dtype=np.float32)
    Wv = np.asarray(inputs["Wv"], dtype=np.float32)
    B = x.shape[0]

    def _cvt(a):
        return np.ascontiguousarray(a).astype(np.float16)

    wqk1 = _cvt(np.concatenate([Wq1, Wk1], axis=1))
    wqk2 = _cvt(np.concatenate([Wq2, Wk2], axis=1))
    wv = _cvt(Wv)
    in_maps = []
    for core in range(8):
        b, h = core // 2, core % 2
        in_maps.append(
            {
                "xt": _cvt(x[b].T),
                "wqk": wqk1 if h == 0 else wqk2,
                "wv": wv,
            }
        )
    return in_maps, B


def _lam(inputs):
    lq1 = np.asarray(inputs["lambda_q1"], dtype=np.float32)
    lk1 = np.asarray(inputs["lambda_k1"], dtype=np.float32)
    lq2 = np.asarray(inputs["lambda_q2"], dtype=np.float32)
    lk2 = np.asarray(inputs["lambda_k2"], dtype=np.float32)
    layer_idx = np.float32(np.asarray(inputs["layer_idx"]))
    dyn_init = np.float32(0.8) - np.float32(0.6) * np.exp(
        np.float32(-0.3) * (layer_idx - np.float32(1.0))
    )
    return np.float32(np.mean(np.exp(lq1 * lk1) - np.exp(lq2 * lk2) + dyn_init))


def _combine(results, lam, B):
    out = np.empty((B, T, E), dtype=np.float32)
    for b in range(B):
        r1, r2 = results[2 * b], results[2 * b + 1]
        l1 = r1["ls"].astype(np.float32).sum(axis=0)  # [T]
        l2 = r2["ls"].astype(np.float32).sum(axis=0)
        o1 = r1["ot"].astype(np.float32) / l1  # [E, T]
        o2 = r2["ot"].astype(np.float32) / l2
        out[b] = (o1 - lam * o2).T
    return out


def run_cores(inputs, loop_n=0, **kwargs):
    """Run the SPMD kernel; returns (BassKernelResults, lam, B)."""
    in_maps, B = _make_in_maps(inputs)
    res = run_bass_kernel_spmd(
        _get_nc(loop_n), in_maps, core_ids=list(range(8)), **kwargs
    )
    return res, _lam(inputs), B


def kernel(**inputs) -> np.ndarray:
    res, lam, B = run_cores(inputs)
    return _combine(res.results, lam, B)

